# revision 9
# baseline (speedup 1.0000x reference)
"""Trainium2 Bass kernel for streaming-decode multi-head self-attention
(Whisper-style, B=128, T_cache=448, D=1280, 20 heads) on 8 NeuronCores.

Sharding: data-parallel over batch (16 samples per core). Each core:
  - projects q/k/v for its samples (q pre-scaled by d^-0.5 on host),
  - streams its K/V cache slices through SBUF once: each tile is written
    back to the shifted output cache position and consumed by attention,
  - scores via PE-transposed K tiles (fp32r matmuls), softmax on ACT/DVE,
  - weighted V sum in natural layout, then the output projection.
"""
import sys

sys.path.insert(0, "/opt/trn_rl_repo")

from contextlib import ExitStack

import numpy as np

import concourse.bass as bass
import concourse.mybir as mybir
import concourse.tile as tile
from concourse.masks import make_identity

# ---------------------------------------------------------------------------
# Workaround: walrus codegen in this container rejects instructions carrying
# more than one sync wait ("Too many sync wait commands"). Hoist extra waits
# onto single-wait NoOp carriers inserted just before, on the same engine.
# ---------------------------------------------------------------------------


def _split_multi_waits(nc):
    for f in nc.m.functions:
        for blk in f.blocks:
            out, changed = [], False
            for inst in blk.instructions:
                si = getattr(inst, "sync_info", None)
                waits = list(si.on_wait) if si is not None else []
                if len(waits) > 1:
                    changed = True
                    for i, w in enumerate(waits[:-1]):
                        out.append(mybir.InstNoOp(
                            name=f"{inst.name}_w{i}",
                            engine=inst.engine,
                            bass_nofuse=True,
                            sync_info=mybir.SyncInfo(on_wait=[w], on_update=[]),
                        ))
                    inst.sync_info = mybir.SyncInfo(
                        on_wait=[waits[-1]], on_update=list(si.on_update))
                out.append(inst)
            if changed:
                blk.instructions = out


# ---------------------------------------------------------------------------

N_CORES = 8
B_FULL, T, D, H = 128, 448, 1280, 20
HD = D // H            # 64 head dim
B = B_FULL // N_CORES  # 16 samples per core
DCH = D // 128         # 10 chunks of 128 dims
F32 = mybir.dt.float32
F32R = mybir.dt.float32r

# shifted-time chunking: t' = 0..447; chunk c covers t' = 128c .. 128c+len-1
TCH_OLD = (128, 128, 128, 63)   # old-cache rows per chunk (t'=447 is the new row)
TCH_ALL = (128, 128, 128, 64)   # including the new row in chunk 3

NSL = [(0, 512), (512, 512), (1024, 256)]


def _r(ap):
    return ap.bitcast(F32R)


def _emit_body(nc, d):
    x_d, kc_in, vc_in, mask_d = d["x"], d["kc_in"], d["vc_in"], d["mask"]
    wq_d, wk_d, wv_d, wo_d = d["Wq"], d["Wk"], d["Wv"], d["Wo"]
    bqT_d, bv_d, bo_d = d["bqT"], d["bv"], d["bo"]
    out_d, kc_out, vc_out = d["out"], d["kc_out"], d["vc_out"]

    with ExitStack() as ctx:
        tc = ctx.enter_context(tile.TileContext(nc))
        persist = ctx.enter_context(tc.tile_pool(name="persist", bufs=1))
        wpool = ctx.enter_context(tc.tile_pool(name="wpool", bufs=DCH))
        kpool = ctx.enter_context(tc.tile_pool(name="kpool", bufs=2))
        vpool = ctx.enter_context(tc.tile_pool(name="vpool", bufs=2))
        ktpool = ctx.enter_context(tc.tile_pool(name="ktpool", bufs=4))
        wspool = ctx.enter_context(tc.tile_pool(name="wspool", bufs=2))
        stpool = ctx.enter_context(tc.tile_pool(name="stpool", bufs=4))

        # ---- constants -----------------------------------------------------
        ident = persist.tile([128, 128], F32)
        make_identity(nc, ident)
        ones = persist.tile([1, 32], F32)
        nc.vector.memset(ones, 1.0)
        mask_sb = persist.tile([1, T], F32)
        nc.sync.dma_start(out=mask_sb, in_=mask_d[:, :])
        bqT = persist.tile([128, DCH], F32)
        nc.sync.dma_start(out=bqT, in_=bqT_d[:, :])
        bv_sb = persist.tile([1, D], F32)
        nc.sync.dma_start(out=bv_sb, in_=bv_d[:, :])
        bo_sb = persist.tile([1, D], F32)
        nc.sync.dma_start(out=bo_sb, in_=bo_d[:, :])

        x_sb = persist.tile([B, D], F32)
        nc.sync.dma_start(out=x_sb, in_=x_d[:, :])

        zeros_f32 = persist.tile([128, DCH * H], F32)
        nc.vector.memset(zeros_f32, 0.0)
        zqd = persist.tile([128, DCH, H], F32R)
        nc.vector.tensor_copy(zqd.rearrange("p c h -> p (c h)"), zeros_f32[:, :])

        xT = persist.tile([128, DCH, B], F32)
        qT = persist.tile([128, DCH, B], F32)
        kT = persist.tile([128, DCH, B], F32)
        k_nat = persist.tile([B, D], F32)
        v_nat = persist.tile([B, D], F32)
        wvT = persist.tile([128, DCH, B], F32)
        out_sb = persist.tile([B, D], F32)

        # ---- phase 1: projections -----------------------------------------
        with tc.tile_pool(name="psum1", bufs=2, space="PSUM") as psum1:
            # x^T [d, b] via PE transposes
            for dc in range(DCH):
                pt = psum1.tile([128, B], F32, tag="pt")
                nc.tensor.transpose(pt[0:128, 0:B], x_sb[0:B, dc * 128:(dc + 1) * 128],
                                    ident[0:B, 0:B])
                nc.scalar.copy(xT[:, dc, :], pt[0:128, 0:B])

            # qT[do, b] = sum_di Wq[di, do] xT[di, b]  (+ bq, host-scaled)
            for w_d, dstT, bias in ((wq_d, qT, bqT), (wk_d, kT, None)):
                w_sb = [wpool.tile([128, D], F32, tag="w", name=f"w{_i}") for _i in range(DCH)]
                for di in range(DCH):
                    nc.sync.dma_start(out=w_sb[di], in_=w_d[di * 128:(di + 1) * 128, :])
                for do in range(DCH):
                    pq = psum1.tile([128, B], F32, tag="pt")
                    for di in range(DCH):
                        nc.tensor.matmul(
                            pq[0:128, 0:B],
                            w_sb[di][:, do * 128:(do + 1) * 128],
                            xT[:, di, :],
                            start=(di == 0), stop=(di == DCH - 1),
                        )
                    if bias is not None:
                        nc.scalar.activation(dstT[:, do, :], pq[0:128, 0:B],
                                             mybir.ActivationFunctionType.Identity,
                                             bias=bias[:, do:do + 1])
                    else:
                        nc.scalar.copy(dstT[:, do, :], pq[0:128, 0:B])
                if w_d is wk_d:
                    # k natural too (cache append row), no bias
                    for (n0, nl) in NSL:
                        pn = psum1.tile([B, 512], F32, tag="pn")
                        for di in range(DCH):
                            nc.tensor.matmul(
                                pn[0:B, 0:nl],
                                xT[:, di, :],
                                w_sb[di][:, n0:n0 + nl],
                                start=(di == 0), stop=(di == DCH - 1),
                            )
                        nc.scalar.copy(k_nat[0:B, n0:n0 + nl], pn[0:B, 0:nl])

            # v natural (+ bv via rank-1 matmul)
            w_sb = [wpool.tile([128, D], F32, tag="w", name=f"w{_i}") for _i in range(DCH)]
            for di in range(DCH):
                nc.sync.dma_start(out=w_sb[di], in_=wv_d[di * 128:(di + 1) * 128, :])
            for (n0, nl) in NSL:
                pn = psum1.tile([B, 512], F32, tag="pn")
                for di in range(DCH):
                    nc.tensor.matmul(pn[0:B, 0:nl], xT[:, di, :],
                                     w_sb[di][:, n0:n0 + nl],
                                     start=(di == 0), stop=False)
                nc.tensor.matmul(pn[0:B, 0:nl], ones[0:1, 0:B],
                                 bv_sb[0:1, n0:n0 + nl], start=False, stop=True)
                nc.scalar.copy(v_nat[0:B, n0:n0 + nl], pn[0:B, 0:nl])

        # ---- phase 2: per-sample cache streaming + attention ---------------
        with tc.tile_pool(name="psum2", bufs=2, space="PSUM") as psum2:
            for b in range(B):
                # K cache load (shifted): t' = old_t - 1
                k_sb = kpool.tile([128, 4, D], F32, tag="k")
                nc.sync.dma_start(
                    out=k_sb[:, 0:3, :],
                    in_=kc_in[b, 1:385, :].rearrange("(c p) d -> p c d", p=128),
                )
                nc.sync.dma_start(out=k_sb[0:63, 3, :], in_=kc_in[b, 385:448, :])
                # shifted store
                nc.sync.dma_start(
                    out=kc_out[b, 0:384, :].rearrange("(c p) d -> p c d", p=128),
                    in_=k_sb[:, 0:3, :],
                )
                nc.sync.dma_start(out=kc_out[b, 384:447, :], in_=k_sb[0:63, 3, :])
                nc.sync.dma_start(out=kc_out[b, 447:448, :], in_=k_nat[b:b + 1, :])

                # V cache load + new row insert + shifted store
                v_sb = vpool.tile([128, 4, D], F32, tag="v")
                nc.sync.dma_start(
                    out=v_sb[:, 0:3, :],
                    in_=vc_in[b, 1:385, :].rearrange("(c p) d -> p c d", p=128),
                )
                nc.sync.dma_start(out=v_sb[0:63, 3, :], in_=vc_in[b, 385:448, :])
                nc.sync.dma_start(out=v_sb[63:64, 3, :], in_=v_nat[b:b + 1, :])
                nc.sync.dma_start(
                    out=vc_out[b, 0:384, :].rearrange("(c p) d -> p c d", p=128),
                    in_=v_sb[:, 0:3, :],
                )
                nc.sync.dma_start(out=vc_out[b, 384:448, :], in_=v_sb[0:64, 3, :])

                # block-diagonal Q for this sample: qd[:, di, h] nonzero only
                # for the head owning dims di*128.. (2 heads per 128-chunk)
                qd = stpool.tile([128, DCH, H], F32R, tag="qd", bufs=2)
                nc.vector.tensor_copy(qd.rearrange("p c h -> p (c h)"),
                                      zqd.rearrange("p c h -> p (c h)"))
                for di in range(DCH):
                    nc.vector.tensor_copy(qd[0:64, di, 2 * di:2 * di + 1],
                                          qT[0:64, di, b:b + 1])
                    nc.vector.tensor_copy(qd[64:128, di, 2 * di + 1:2 * di + 2],
                                          qT[64:128, di, b:b + 1])

                # scores psum [H, T]: accumulate over 10 dim-chunks, + mask
                ps = psum2.tile([H, T], F32, tag="sc")
                for di in range(DCH):
                    kt = ktpool.tile([128, T], F32R, tag="kt")
                    for c in range(4):
                        tl = TCH_OLD[c]
                        pt = psum2.tile([128, 128], F32, tag="tp")
                        nc.tensor.transpose(
                            pt[0:128, 0:tl],
                            k_sb[0:tl, c, di * 128:(di + 1) * 128],
                            ident[0:tl, 0:tl],
                        )
                        nc.vector.tensor_copy(kt[:, c * 128:c * 128 + tl],
                                              pt[0:128, 0:tl])
                    nc.vector.tensor_copy(kt[:, 447:448], kT[:, di, b:b + 1])
                    nc.tensor.matmul(ps[0:H, :], qd[:, di, :], kt[:, :],
                                     start=(di == 0), stop=False)
                # + mask (rank-1 accumulate, broadcast over heads)
                nc.tensor.matmul(ps[0:H, :], ones[0:1, 0:H], mask_sb[0:1, :],
                                 start=False, stop=True)

                # softmax over t (free dim)
                negmax = stpool.tile([H, 1], F32, tag="nm")
                nc.vector.reduce_max(out=negmax, in_=ps[:, :],
                                     axis=mybir.AxisListType.X, negate=True)
                w_sb2 = wspool.tile([H, T], F32, tag="ws")
                ssum = stpool.tile([H, 1], F32, tag="ss")
                nc.scalar.activation(w_sb2[:, :], ps[:, :],
                                     mybir.ActivationFunctionType.Exp,
                                     bias=negmax, accum_out=ssum)
                rcp = stpool.tile([H, 1], F32, tag="rc")
                nc.vector.reciprocal(rcp, ssum)
                nc.vector.tensor_scalar_mul(w_sb2[:, :], w_sb2[:, :], rcp)

                # w^T [t, h] chunks
                wT = wspool.tile([128, 4, H], F32, tag="wt")
                for c in range(4):
                    tl = TCH_ALL[c]
                    pw = psum2.tile([128, H], F32, tag="wp")
                    nc.tensor.transpose(pw[0:tl, 0:H],
                                        w_sb2[0:H, c * 128:c * 128 + tl],
                                        ident[0:H, 0:H])
                    nc.scalar.copy(wT[0:tl, c, :], pw[0:tl, 0:H])

                # wv^T via per-128-dim-chunk matmuls (2 heads per chunk)
                for dc in range(DCH):
                    pv = psum2.tile([128, H], F32, tag="wv")
                    for c in range(4):
                        tl = TCH_ALL[c]
                        nc.tensor.matmul(
                            pv[0:128, 0:H],
                            v_sb[0:tl, c, dc * 128:(dc + 1) * 128],
                            wT[0:tl, c, :],
                            start=(c == 0), stop=(c == 3),
                        )
                    h0, h1 = 2 * dc, 2 * dc + 1
                    nc.vector.tensor_copy(wvT[0:64, dc, b:b + 1], pv[0:64, h0:h0 + 1])
                    nc.vector.tensor_copy(wvT[64:128, dc, b:b + 1],
                                          pv[64:128, h1:h1 + 1])

        # ---- phase 3: output projection ------------------------------------
        w_sb = [wpool.tile([128, D], F32, tag="w", name=f"w{_i}") for _i in range(DCH)]
        for di in range(DCH):
            nc.sync.dma_start(out=w_sb[di], in_=wo_d[di * 128:(di + 1) * 128, :])
        with tc.tile_pool(name="psum3", bufs=2, space="PSUM") as psum3:
            for (n0, nl) in NSL:
                po = psum3.tile([B, 512], F32, tag="po")
                for di in range(DCH):
                    nc.tensor.matmul(po[0:B, 0:nl], wvT[:, di, :],
                                     w_sb[di][:, n0:n0 + nl],
                                     start=(di == 0), stop=False)
                nc.tensor.matmul(po[0:B, 0:nl], ones[0:1, 0:B],
                                 bo_sb[0:1, n0:n0 + nl], start=False, stop=True)
                nc.scalar.copy(out_sb[0:B, n0:n0 + nl], po[0:B, 0:nl])
        nc.sync.dma_start(out=out_d[:, :], in_=out_sb[:, :])


_NC = None


def _get_nc():
    global _NC
    if _NC is None:
        nc = bass.Bass()
        d = {}
        for name, shape in [
            ("x", [B, D]), ("kc_in", [B, T, D]), ("vc_in", [B, T, D]),
            ("mask", [1, T]), ("Wq", [D, D]), ("Wk", [D, D]), ("Wv", [D, D]),
            ("Wo", [D, D]), ("bqT", [128, DCH]), ("bv", [1, D]), ("bo", [1, D]),
        ]:
            d[name] = nc.dram_tensor(name, shape, F32, kind="ExternalInput")
        for name, shape in [
            ("out", [B, D]), ("kc_out", [B, T, D]), ("vc_out", [B, T, D]),
        ]:
            d[name] = nc.dram_tensor(name, shape, F32, kind="ExternalOutput")
        _emit_body(nc, d)
        _split_multi_waits(nc)
        _NC = nc
    return _NC


def kernel(x, k_cache, v_cache, mask, Wq, bq, Wk, Wv, bv, Wo, bo, _trace=False):
    from concourse.bass_utils import run_bass_kernel_spmd

    x = np.asarray(x, dtype=np.float32).reshape(B_FULL, D)
    k_cache = np.ascontiguousarray(np.asarray(k_cache, dtype=np.float32))
    v_cache = np.ascontiguousarray(np.asarray(v_cache, dtype=np.float32))
    mask_a = np.asarray(mask, dtype=np.float32).reshape(1, T)
    scale = float(HD) ** -0.5
    Wq_s = np.ascontiguousarray(np.asarray(Wq, dtype=np.float32) * scale)
    bq_s = np.asarray(bq, dtype=np.float32) * scale
    bqT = np.ascontiguousarray(bq_s.reshape(DCH, 128).T)
    Wk_a = np.ascontiguousarray(np.asarray(Wk, dtype=np.float32))
    Wv_a = np.ascontiguousarray(np.asarray(Wv, dtype=np.float32))
    Wo_a = np.ascontiguousarray(np.asarray(Wo, dtype=np.float32))
    bv_a = np.asarray(bv, dtype=np.float32).reshape(1, D)
    bo_a = np.asarray(bo, dtype=np.float32).reshape(1, D)

    nc = _get_nc()
    in_maps = []
    for c in range(N_CORES):
        sl = slice(c * B, (c + 1) * B)
        in_maps.append({
            "x": x[sl], "kc_in": k_cache[sl], "vc_in": v_cache[sl],
            "mask": mask_a, "Wq": Wq_s, "Wk": Wk_a, "Wv": Wv_a, "Wo": Wo_a,
            "bqT": bqT, "bv": bv_a, "bo": bo_a,
        })
    res = run_bass_kernel_spmd(nc, in_maps, list(range(N_CORES)), trace=_trace)
    out = np.concatenate([res.results[c]["out"] for c in range(N_CORES)], axis=0)
    kc = np.concatenate([res.results[c]["kc_out"] for c in range(N_CORES)], axis=0)
    vc = np.concatenate([res.results[c]["vc_out"] for c in range(N_CORES)], axis=0)
    out = out.reshape(B_FULL, 1, D)
    if _trace:
        kernel.last_results = res
    return out, kc, vc


# revision 10
# speedup vs baseline: 86.1114x; 86.1114x over previous
"""Trainium2 Bass kernel for streaming-decode multi-head self-attention
(Whisper-style, B=128, T_cache=448, D=1280, 20 heads) on 8 NeuronCores.

Sharding: data-parallel over batch (16 samples per core). Each core:
  - projects q/k/v for its samples (q pre-scaled by d^-0.5 on host),
  - streams its K/V cache slices through SBUF once: each tile is written
    back to the shifted output cache position and consumed by attention,
  - scores via PE-transposed K tiles (fp32r matmuls), softmax on ACT/DVE,
  - weighted V sum in natural layout, then the output projection.
"""
import sys

sys.path.insert(0, "/opt/trn_rl_repo")

from contextlib import ExitStack

import numpy as np

import concourse.bass as bass
import concourse.mybir as mybir
import concourse.tile as tile
from concourse.masks import make_identity

# ---------------------------------------------------------------------------
# Workaround: walrus codegen in this container rejects instructions carrying
# more than one sync wait ("Too many sync wait commands"). Hoist extra waits
# onto single-wait NoOp carriers inserted just before, on the same engine.
# ---------------------------------------------------------------------------


def _split_multi_waits(nc):
    for f in nc.m.functions:
        for blk in f.blocks:
            out, changed = [], False
            for inst in blk.instructions:
                si = getattr(inst, "sync_info", None)
                waits = list(si.on_wait) if si is not None else []
                if len(waits) > 1:
                    changed = True
                    for i, w in enumerate(waits[:-1]):
                        out.append(mybir.InstNoOp(
                            name=f"{inst.name}_w{i}",
                            engine=inst.engine,
                            bass_nofuse=True,
                            sync_info=mybir.SyncInfo(on_wait=[w], on_update=[]),
                        ))
                    inst.sync_info = mybir.SyncInfo(
                        on_wait=[waits[-1]], on_update=list(si.on_update))
                out.append(inst)
            if changed:
                blk.instructions = out


# ---------------------------------------------------------------------------

N_CORES = 8
B_FULL, T, D, H = 128, 448, 1280, 20
HD = D // H            # 64 head dim
B = B_FULL // N_CORES  # 16 samples per core
DCH = D // 128         # 10 chunks of 128 dims
F32 = mybir.dt.float32
F32R = mybir.dt.float32r

# shifted-time chunking: t' = 0..447; chunk c covers t' = 128c .. 128c+len-1
TCH_OLD = (128, 128, 128, 63)   # old-cache rows per chunk (t'=447 is the new row)
TCH_ALL = (128, 128, 128, 64)   # including the new row in chunk 3

NSL = [(0, 512), (512, 512), (1024, 256)]


def _r(ap):
    return ap.bitcast(F32R)


def _emit_body(nc, d):
    x_d, kc_in, vc_in, mask_d = d["x"], d["kc_in"], d["vc_in"], d["mask"]
    wq_d, wk_d, wv_d, wo_d = d["Wq"], d["Wk"], d["Wv"], d["Wo"]
    bqT_d, bv_d, bo_d = d["bqT"], d["bv"], d["bo"]
    out_d, kc_out, vc_out = d["out"], d["kc_out"], d["vc_out"]

    with ExitStack() as ctx:
        tc = ctx.enter_context(tile.TileContext(nc))
        persist = ctx.enter_context(tc.tile_pool(name="persist", bufs=1))
        wpool = ctx.enter_context(tc.tile_pool(name="wpool", bufs=DCH))
        kpool = ctx.enter_context(tc.tile_pool(name="kpool", bufs=2))
        vpool = ctx.enter_context(tc.tile_pool(name="vpool", bufs=2))
        ktpool = ctx.enter_context(tc.tile_pool(name="ktpool", bufs=4))
        wspool = ctx.enter_context(tc.tile_pool(name="wspool", bufs=2))
        stpool = ctx.enter_context(tc.tile_pool(name="stpool", bufs=4))

        # ---- constants -----------------------------------------------------
        ident = persist.tile([128, 128], F32)
        make_identity(nc, ident)
        ones = persist.tile([1, 32], F32)
        nc.vector.memset(ones, 1.0)
        ones_r = persist.tile([1, 32], F32R)
        nc.vector.tensor_copy(ones_r[:, :], ones[:, :])
        mask_sb = persist.tile([1, T], F32)
        nc.sync.dma_start(out=mask_sb, in_=mask_d[:, :])
        mask_r = persist.tile([1, T], F32R)
        nc.vector.tensor_copy(mask_r[:, :], mask_sb[:, :])
        bqT = persist.tile([128, DCH], F32)
        nc.sync.dma_start(out=bqT, in_=bqT_d[:, :])
        bv_sb = persist.tile([1, D], F32)
        nc.sync.dma_start(out=bv_sb, in_=bv_d[:, :])
        bo_sb = persist.tile([1, D], F32)
        nc.sync.dma_start(out=bo_sb, in_=bo_d[:, :])

        x_sb = persist.tile([B, D], F32)
        nc.sync.dma_start(out=x_sb, in_=x_d[:, :])

        zeros_f32 = persist.tile([128, DCH * H], F32)
        nc.vector.memset(zeros_f32, 0.0)
        zqd = persist.tile([128, DCH, H], F32R)
        nc.vector.tensor_copy(zqd.rearrange("p c h -> p (c h)"), zeros_f32[:, :])

        xT = persist.tile([128, DCH, B], F32)
        qT = persist.tile([128, DCH, B], F32)
        kT = persist.tile([128, DCH, B], F32)
        k_nat = persist.tile([B, D], F32)
        v_nat = persist.tile([B, D], F32)
        wvT = persist.tile([128, DCH, B], F32)
        out_sb = persist.tile([B, D], F32)

        # ---- phase 1: projections -----------------------------------------
        with tc.tile_pool(name="psum1", bufs=2, space="PSUM") as psum1:
            # x^T [d, b] via PE transposes
            for dc in range(DCH):
                pt = psum1.tile([128, B], F32, tag="pt")
                nc.tensor.transpose(pt[0:128, 0:B], x_sb[0:B, dc * 128:(dc + 1) * 128],
                                    ident[0:B, 0:B])
                nc.scalar.copy(xT[:, dc, :], pt[0:128, 0:B])

            # qT[do, b] = sum_di Wq[di, do] xT[di, b]  (+ bq, host-scaled)
            for w_d, dstT, bias in ((wq_d, qT, bqT), (wk_d, kT, None)):
                w_sb = [wpool.tile([128, D], F32, tag="w", name=f"w{_i}") for _i in range(DCH)]
                for di in range(DCH):
                    nc.sync.dma_start(out=w_sb[di], in_=w_d[di * 128:(di + 1) * 128, :])
                for do in range(DCH):
                    pq = psum1.tile([128, B], F32, tag="pt")
                    for di in range(DCH):
                        nc.tensor.matmul(
                            pq[0:128, 0:B],
                            w_sb[di][:, do * 128:(do + 1) * 128],
                            xT[:, di, :],
                            start=(di == 0), stop=(di == DCH - 1),
                        )
                    if bias is not None:
                        nc.scalar.activation(dstT[:, do, :], pq[0:128, 0:B],
                                             mybir.ActivationFunctionType.Identity,
                                             bias=bias[:, do:do + 1])
                    else:
                        nc.scalar.copy(dstT[:, do, :], pq[0:128, 0:B])
                if w_d is wk_d:
                    # k natural too (cache append row), no bias
                    for (n0, nl) in NSL:
                        pn = psum1.tile([B, 512], F32, tag="pn")
                        for di in range(DCH):
                            nc.tensor.matmul(
                                pn[0:B, 0:nl],
                                xT[:, di, :],
                                w_sb[di][:, n0:n0 + nl],
                                start=(di == 0), stop=(di == DCH - 1),
                            )
                        nc.scalar.copy(k_nat[0:B, n0:n0 + nl], pn[0:B, 0:nl])

            # v natural (+ bv via rank-1 matmul)
            w_sb = [wpool.tile([128, D], F32, tag="w", name=f"w{_i}") for _i in range(DCH)]
            for di in range(DCH):
                nc.sync.dma_start(out=w_sb[di], in_=wv_d[di * 128:(di + 1) * 128, :])
            for (n0, nl) in NSL:
                pn = psum1.tile([B, 512], F32, tag="pn")
                for di in range(DCH):
                    nc.tensor.matmul(pn[0:B, 0:nl], xT[:, di, :],
                                     w_sb[di][:, n0:n0 + nl],
                                     start=(di == 0), stop=False)
                nc.tensor.matmul(pn[0:B, 0:nl], ones[0:1, 0:B],
                                 bv_sb[0:1, n0:n0 + nl], start=False, stop=True)
                nc.scalar.copy(v_nat[0:B, n0:n0 + nl], pn[0:B, 0:nl])

        # ---- phase 2: per-sample cache streaming + attention ---------------
        with tc.tile_pool(name="psum2", bufs=2, space="PSUM") as psum2:
            for b in range(B):
                # K cache load (shifted): t' = old_t - 1
                k_sb = kpool.tile([128, 4, D], F32, tag="k")
                nc.sync.dma_start(
                    out=k_sb[:, 0:3, :],
                    in_=kc_in[b, 1:385, :].rearrange("(c p) d -> p c d", p=128),
                )
                nc.sync.dma_start(out=k_sb[0:63, 3, :], in_=kc_in[b, 385:448, :])
                # shifted store
                nc.sync.dma_start(
                    out=kc_out[b, 0:384, :].rearrange("(c p) d -> p c d", p=128),
                    in_=k_sb[:, 0:3, :],
                )
                nc.sync.dma_start(out=kc_out[b, 384:447, :], in_=k_sb[0:63, 3, :])
                nc.sync.dma_start(out=kc_out[b, 447:448, :], in_=k_nat[b:b + 1, :])

                # V cache load + new row insert + shifted store
                v_sb = vpool.tile([128, 4, D], F32, tag="v")
                nc.sync.dma_start(
                    out=v_sb[:, 0:3, :],
                    in_=vc_in[b, 1:385, :].rearrange("(c p) d -> p c d", p=128),
                )
                nc.sync.dma_start(out=v_sb[0:63, 3, :], in_=vc_in[b, 385:448, :])
                nc.sync.dma_start(out=v_sb[63:64, 3, :], in_=v_nat[b:b + 1, :])
                nc.sync.dma_start(
                    out=vc_out[b, 0:384, :].rearrange("(c p) d -> p c d", p=128),
                    in_=v_sb[:, 0:3, :],
                )
                nc.sync.dma_start(out=vc_out[b, 384:448, :], in_=v_sb[0:64, 3, :])

                # block-diagonal Q for this sample: qd[:, di, h] nonzero only
                # for the head owning dims di*128.. (2 heads per 128-chunk)
                qd = stpool.tile([128, DCH, H], F32R, tag="qd", bufs=2)
                nc.vector.tensor_copy(qd.rearrange("p c h -> p (c h)"),
                                      zqd.rearrange("p c h -> p (c h)"))
                for di in range(DCH):
                    nc.vector.tensor_copy(qd[0:64, di, 2 * di:2 * di + 1],
                                          qT[0:64, di, b:b + 1])
                    nc.vector.tensor_copy(qd[64:128, di, 2 * di + 1:2 * di + 2],
                                          qT[64:128, di, b:b + 1])

                # scores psum [H, T]: accumulate over 10 dim-chunks, + mask
                ps = psum2.tile([H, T], F32, tag="sc")
                for di in range(DCH):
                    kt = ktpool.tile([128, T], F32R, tag="kt")
                    for c in range(4):
                        tl = TCH_OLD[c]
                        pt = psum2.tile([128, 128], F32, tag="tp")
                        nc.tensor.transpose(
                            pt[0:128, 0:tl],
                            k_sb[0:tl, c, di * 128:(di + 1) * 128],
                            ident[0:tl, 0:tl],
                        )
                        nc.vector.tensor_copy(kt[:, c * 128:c * 128 + tl],
                                              pt[0:128, 0:tl])
                    nc.vector.tensor_copy(kt[:, 447:448], kT[:, di, b:b + 1])
                    nc.tensor.matmul(ps[0:H, :], qd[:, di, :], kt[:, :],
                                     start=(di == 0), stop=False)
                # + mask (rank-1 accumulate, broadcast over heads)
                nc.tensor.matmul(ps[0:H, :], ones_r[0:1, 0:H], mask_r[0:1, :],
                                 start=False, stop=True)

                # softmax over t (free dim)
                negmax = stpool.tile([H, 1], F32, tag="nm")
                nc.vector.reduce_max(out=negmax, in_=ps[:, :],
                                     axis=mybir.AxisListType.X, negate=True)
                w_sb2 = wspool.tile([H, T], F32, tag="ws")
                ssum = stpool.tile([H, 1], F32, tag="ss")
                nc.scalar.activation(w_sb2[:, :], ps[:, :],
                                     mybir.ActivationFunctionType.Exp,
                                     bias=negmax, accum_out=ssum)
                rcp = stpool.tile([H, 1], F32, tag="rc")
                nc.vector.reciprocal(rcp, ssum)
                nc.vector.tensor_scalar_mul(w_sb2[:, :], w_sb2[:, :], rcp)

                # w^T [t, h] chunks
                wT = wspool.tile([128, 4, H], F32, tag="wt")
                for c in range(4):
                    tl = TCH_ALL[c]
                    pw = psum2.tile([128, H], F32, tag="wp")
                    nc.tensor.transpose(pw[0:tl, 0:H],
                                        w_sb2[0:H, c * 128:c * 128 + tl],
                                        ident[0:H, 0:H])
                    nc.scalar.copy(wT[0:tl, c, :], pw[0:tl, 0:H])

                # wv^T via per-128-dim-chunk matmuls (2 heads per chunk)
                for dc in range(DCH):
                    pv = psum2.tile([128, H], F32, tag="wv")
                    for c in range(4):
                        tl = TCH_ALL[c]
                        nc.tensor.matmul(
                            pv[0:128, 0:H],
                            v_sb[0:tl, c, dc * 128:(dc + 1) * 128],
                            wT[0:tl, c, :],
                            start=(c == 0), stop=(c == 3),
                        )
                    h0, h1 = 2 * dc, 2 * dc + 1
                    nc.vector.tensor_copy(wvT[0:64, dc, b:b + 1], pv[0:64, h0:h0 + 1])
                    nc.vector.tensor_copy(wvT[64:128, dc, b:b + 1],
                                          pv[64:128, h1:h1 + 1])

        # ---- phase 3: output projection ------------------------------------
        w_sb = [wpool.tile([128, D], F32, tag="w", name=f"w{_i}") for _i in range(DCH)]
        for di in range(DCH):
            nc.sync.dma_start(out=w_sb[di], in_=wo_d[di * 128:(di + 1) * 128, :])
        with tc.tile_pool(name="psum3", bufs=2, space="PSUM") as psum3:
            for (n0, nl) in NSL:
                po = psum3.tile([B, 512], F32, tag="po")
                for di in range(DCH):
                    nc.tensor.matmul(po[0:B, 0:nl], wvT[:, di, :],
                                     w_sb[di][:, n0:n0 + nl],
                                     start=(di == 0), stop=False)
                nc.tensor.matmul(po[0:B, 0:nl], ones[0:1, 0:B],
                                 bo_sb[0:1, n0:n0 + nl], start=False, stop=True)
                nc.scalar.copy(out_sb[0:B, n0:n0 + nl], po[0:B, 0:nl])
        nc.sync.dma_start(out=out_d[:, :], in_=out_sb[:, :])


_NC = None


def _get_nc():
    global _NC
    if _NC is None:
        nc = bass.Bass()
        d = {}
        for name, shape in [
            ("x", [B, D]), ("kc_in", [B, T, D]), ("vc_in", [B, T, D]),
            ("mask", [1, T]), ("Wq", [D, D]), ("Wk", [D, D]), ("Wv", [D, D]),
            ("Wo", [D, D]), ("bqT", [128, DCH]), ("bv", [1, D]), ("bo", [1, D]),
        ]:
            d[name] = nc.dram_tensor(name, shape, F32, kind="ExternalInput")
        for name, shape in [
            ("out", [B, D]), ("kc_out", [B, T, D]), ("vc_out", [B, T, D]),
        ]:
            d[name] = nc.dram_tensor(name, shape, F32, kind="ExternalOutput")
        _emit_body(nc, d)
        _split_multi_waits(nc)
        _NC = nc
    return _NC


def kernel(x, k_cache, v_cache, mask, Wq, bq, Wk, Wv, bv, Wo, bo, _trace=False):
    from concourse.bass_utils import run_bass_kernel_spmd

    x = np.asarray(x, dtype=np.float32).reshape(B_FULL, D)
    k_cache = np.ascontiguousarray(np.asarray(k_cache, dtype=np.float32))
    v_cache = np.ascontiguousarray(np.asarray(v_cache, dtype=np.float32))
    mask_a = np.asarray(mask, dtype=np.float32).reshape(1, T)
    scale = float(HD) ** -0.5
    Wq_s = np.ascontiguousarray(np.asarray(Wq, dtype=np.float32) * scale)
    bq_s = np.asarray(bq, dtype=np.float32) * scale
    bqT = np.ascontiguousarray(bq_s.reshape(DCH, 128).T)
    Wk_a = np.ascontiguousarray(np.asarray(Wk, dtype=np.float32))
    Wv_a = np.ascontiguousarray(np.asarray(Wv, dtype=np.float32))
    Wo_a = np.ascontiguousarray(np.asarray(Wo, dtype=np.float32))
    bv_a = np.asarray(bv, dtype=np.float32).reshape(1, D)
    bo_a = np.asarray(bo, dtype=np.float32).reshape(1, D)

    nc = _get_nc()
    in_maps = []
    for c in range(N_CORES):
        sl = slice(c * B, (c + 1) * B)
        in_maps.append({
            "x": x[sl], "kc_in": k_cache[sl], "vc_in": v_cache[sl],
            "mask": mask_a, "Wq": Wq_s, "Wk": Wk_a, "Wv": Wv_a, "Wo": Wo_a,
            "bqT": bqT, "bv": bv_a, "bo": bo_a,
        })
    res = run_bass_kernel_spmd(nc, in_maps, list(range(N_CORES)), trace=_trace)
    out = np.concatenate([res.results[c]["out"] for c in range(N_CORES)], axis=0)
    kc = np.concatenate([res.results[c]["kc_out"] for c in range(N_CORES)], axis=0)
    vc = np.concatenate([res.results[c]["vc_out"] for c in range(N_CORES)], axis=0)
    out = out.reshape(B_FULL, 1, D)
    if _trace:
        kernel.last_results = res
    return out, kc, vc
